# revision 69
# baseline (speedup 1.0000x reference)
"""TRN2 Bass kernel for nn_CausalSelfAttention_63058709840004.

Sharding: tensor-parallel over heads (2 groups x 3 heads) x 4 causal query
shards = 8 cores. Each core projects K,V for one sequence QUARTER of its 3
heads (quarter = QOF[s], so low k-tiles land on the cores that finish their
gather wait last), AllGathers K^T/V within its 4-core group through
a DRAM bounce blob, projects Q for its 1024 query rows (two 512-row chunks
at s*512 and (7-s)*512), runs causal attention with the own quarter read
pre-gather from SBUF, and a partial c_proj; the host sums the two
head-group partials per row.

Mostly bf16 (x, weights, K^T/Q^T/V/E tiles, rotary tables, c_proj, output)
with fp32 PSUM accumulation everywhere and fp32 softmax statistics. ssq is
computed from the pre-rotary values (rotation preserves row norms) with a
DVE square + segmented reduce, so the sqrt/recip chain overlaps the rotary
ops; rows are then rms-scaled in one all-bf16 DVE op (K gets 1/rms, Q gets
ATTN_SCALE/rms). V's PSUM is freed through a fast Act copy with the ve add
deferred to an all-bf16 DVE op. Causal diagonal blocks use sub-range
matmuls + narrow [128,128] affine_selects. Softmax skips max-subtraction
(|scores| <= 15.4 bounded after rms-norm, safe in fp32). The softmax
denominator rides PSUM via ones-column matmuls (exact partition
reduction). A compiled shard_map runner is cached so repeat kernel()
calls skip retracing.
"""
import numpy as np
import ml_dtypes

import concourse.bass as bass
import concourse.bacc as bacc
import concourse.mybir as mybir
import concourse.tile as tile
from concourse.bass_utils import run_bass_kernel_spmd

T, DIM, H, D = 4096, 768, 6, 128
HPG = 3  # heads per group
GDIM = HPG * D  # 384
ATTN_SCALE = 0.12
EPS = 1.1920929e-07
NT = T // 128  # 32 t-tiles
NQ = 1024 // 128  # 8 q-tiles per core
F32 = mybir.dt.float32
F32R = mybir.dt.float32r
BF16 = mybir.dt.bfloat16
U32 = mybir.dt.uint32
EXP = mybir.ActivationFunctionType.Exp
SQRT = mybir.ActivationFunctionType.Sqrt
COPY = mybir.ActivationFunctionType.Copy
NPBF16 = ml_dtypes.bfloat16

_CACHE = {}

DEBUG = False  # add intermediate-dump outputs
DEDUP = True  # each core projects K,V for T/4 only, AllGather within group
NKVP = (NT // 4 if DEDUP else NT) // 2  # xTt pair count per core
QOF = [3, 2, 1, 0]  # gather slot i (device s=i in group) holds quarter QOF[i]


def _rotary_u(nc, pool, nat_u, cos_u, sin_u):
    """In-place rotary on nat_u [128, HPG, 128] bf16 (one t-tile).

    Rotates dim pairs (i, 64+i), i in 0..31 (freqs 32..63 are zero ->
    identity). cos_u: [128, 32] table slice; sin_u: [128, 64] = [s | -s].
    All operand APs stay <= 3 free dims (walrus TENSOR3D limit).
    """
    x12 = nat_u[:, :, 0:128].rearrange("p h (a b) -> p h a b", b=32)[:, :, 0:3:2]
    x1 = nat_u[:, :, 0:32]
    x2 = nat_u[:, :, 64:96]
    u2 = pool.tile([128, HPG, 2, 32], BF16, tag="rot_u")
    t2 = pool.tile([128, HPG, 2, 32], BF16, tag="rot_t")
    cb = cos_u[:, None, None, :].to_broadcast((128, HPG, 2, 32))
    sp = sin_u[:, None, 0:32].to_broadcast((128, HPG, 32))
    sn = sin_u[:, None, 32:64].to_broadcast((128, HPG, 32))
    # u = (x1, x2) * cos ; t = (x2*s, x1*(-s))
    nc.vector.tensor_mul(out=u2[:], in0=x12, in1=cb)
    nc.vector.tensor_mul(out=t2[:, :, 0], in0=x2, in1=sp)
    nc.vector.tensor_mul(out=t2[:, :, 1], in0=x1, in1=sn)
    # (y1, y2) = u + t
    nc.vector.tensor_add(out=x12, in0=u2[:], in1=t2[:])


def build_nc(variant=None, sim_cc=False):
    nc = bacc.Bacc(None, target_bir_lowering=False, num_devices=8)

    # ---- DRAM tensors (per-core inputs prepared by the host) ----
    xTt = nc.dram_tensor("xTt", [NKVP, 128, 6, 2, 128], BF16, kind="ExternalInput")
    xqTt = nc.dram_tensor("xqTt", [NQ // 2, 128, 6, 2, 128], BF16, kind="ExternalInput")
    wkv = nc.dram_tensor("wkv", [128, 6, 2 * GDIM], BF16, kind="ExternalInput")
    wq = nc.dram_tensor("wq", [128, 6, GDIM], BF16, kind="ExternalInput")
    vek = nc.dram_tensor("vek", [NKVP, 128, 2, GDIM], BF16, kind="ExternalInput")
    NKT = 2 * NKVP  # locally projected K/V tiles
    cosk = nc.dram_tensor("cosk", [128, NKT, 32], BF16, kind="ExternalInput")
    sinkpm = nc.dram_tensor("sinkpm", [128, NKT, 64], BF16, kind="ExternalInput")
    cosq = nc.dram_tensor("cosq", [128, NQ, 32], BF16, kind="ExternalInput")
    sinqpm = nc.dram_tensor("sinqpm", [128, NQ, 64], BF16, kind="ExternalInput")
    cprojT = nc.dram_tensor("cprojT", [128, HPG, DIM], BF16, kind="ExternalInput")
    ident_in = nc.dram_tensor("ident", [128, 128], BF16, kind="ExternalInput")
    ones_col_in = nc.dram_tensor("ones_col", [128, 1], BF16, kind="ExternalInput")
    svar_t = nc.dram_tensor("svar", [1, 1], U32, kind="ExternalInput")
    y_out = nc.dram_tensor("y", [1024, DIM], BF16, kind="ExternalOutput")
    if DEBUG:
        dbg_kt = nc.dram_tensor("dbg_kt", [128, HPG, T], BF16, kind="ExternalOutput")
        dbg_qt = nc.dram_tensor("dbg_qt", [128, HPG, 1024], BF16, kind="ExternalOutput")
        dbg_bs = nc.dram_tensor("dbg_bs", [128, HPG, NT], F32, kind="ExternalOutput")
        dbg_vn = nc.dram_tensor("dbg_vn", [128, NT, GDIM], F32, kind="ExternalOutput")
        dbg_ys = nc.dram_tensor("dbg_ys", [128, HPG, 1024], F32, kind="ExternalOutput")

    with tile.TileContext(nc) as tc:
        with tc.tile_pool(name="res", bufs=1) as res:
            KT = res.tile([128, HPG, T], BF16, tag="KT")
            Vn = res.tile([128, NT, GDIM], BF16, tag="Vn")
            QT = res.tile([128, HPG, 1024], BF16, tag="QT")
            Ysb = res.tile([128, HPG, 1024], BF16, tag="Ysb")
            if DEDUP:
                KTq = res.tile([128, HPG, 128 * NKT], BF16, tag="KTq")
                Vq = res.tile([128, NKT, GDIM], BF16, tag="Vq")
            cproj_sb = res.tile([128, HPG, DIM], BF16, tag="cproj")
            ident = res.tile([128, 128], BF16, tag="ident")
            ones_col = res.tile([128, 1], BF16, tag="ones_col")
            nc.gpsimd.dma_start(ident[:], ident_in[:])
            nc.gpsimd.dma_start(ones_col[:], ones_col_in[:])
            eps_k = res.tile([128, 1], F32, tag="eps_k")
            eps_q = res.tile([128, 1], F32, tag="eps_q")
            nc.gpsimd.memset(eps_k[:], EPS)
            nc.gpsimd.memset(eps_q[:], EPS / (ATTN_SCALE * ATTN_SCALE))

            # ================= Phase A/B: projections =================
            with (
                tc.tile_pool(name="wp", bufs=1) as wp,
                tc.tile_pool(name="ap", bufs=4) as ap,
                tc.tile_pool(name="st", bufs=4) as st,
                tc.tile_pool(name="rot", bufs=2) as rot,
                tc.tile_pool(name="pp", bufs=3, space="PSUM") as pp,
                tc.tile_pool(name="pt", bufs=2, space="PSUM") as pt,
            ):
                wkv_sb = wp.tile([128, 6, 2 * GDIM], BF16, tag="wkv")
                wq_sb = wp.tile([128, 6, GDIM], BF16, tag="wq")
                cosk_sb = wp.tile([128, NKT, 32], BF16, tag="cosk")
                sinkpm_sb = wp.tile([128, NKT, 64], BF16, tag="sinkpm")
                cosq_sb = wp.tile([128, NQ, 32], BF16, tag="cosq")
                sinqpm_sb = wp.tile([128, NQ, 64], BF16, tag="sinqpm")
                for md in range(6):
                    nc.scalar.dma_start(wkv_sb[:, md], wkv[:, md])
                    nc.scalar.dma_start(wq_sb[:, md], wq[:, md])
                nc.scalar.dma_start(cosk_sb[:], cosk[:])
                nc.scalar.dma_start(sinkpm_sb[:], sinkpm[:])
                nc.scalar.dma_start(cosq_sb[:], cosq[:])
                nc.scalar.dma_start(sinqpm_sb[:], sinqpm[:])

                pending_tr = []  # delayed transposes: (nat2, is_q, tp)

                def flush_tr(n=99):
                    for _ in range(min(n, len(pending_tr))):
                        nat2, is_q, tp, w = pending_tr.pop(0)
                        dst = QT if is_q else (KTq if DEDUP else KT)
                        for u in range(w):
                            ti = w * tp + u
                            tr = pt.tile([128, GDIM], BF16, tag="tr")
                            for h in range(HPG):
                                nc.tensor.transpose(
                                    tr[:, h * D : (h + 1) * D], nat2[:, u, h], ident[:]
                                )
                            nc.vector.tensor_copy(
                                dst[:, :, ti * 128 : (ti + 1) * 128],
                                tr[:].rearrange("p (h d) -> p h d", d=D),
                            )

                def proj_pair(tp, is_q, w=2):
                    """Project, rotate, rms-stat, transpose w 128-row tiles."""
                    src = xqTt if is_q else xTt
                    wmat = wq_sb if is_q else wkv_sb
                    xt2 = st.tile([128, 6, w, 128], BF16, tag=f"xt{w}")
                    nc.sync.dma_start(xt2[:], src[tp * w // 2 : (tp + 1) * w // 2])
                    nat2 = ap.tile([128, w, HPG, D], BF16, tag=f"nat{w}")
                    if not is_q:
                        vet2 = st.tile([128, w, GDIM], BF16, tag="vet")
                        nc.gpsimd.dma_start(vet2[:], vek[tp])
                        vraw2 = ap.tile([128, w, GDIM], BF16, tag="vraw")
                    for u in range(w):
                        ti = w * tp + u
                        k_ps = pp.tile([128, GDIM], F32, tag="k_ps")
                        if not is_q:
                            v_ps = pp.tile([128, GDIM], F32, tag="v_ps")
                        for md in range(6):
                            nc.tensor.matmul(
                                k_ps[:], xt2[:, md, u], wmat[:, md, 0:GDIM],
                                start=(md == 0), stop=(md == 5), skip_group_check=True,
                            )
                            if not is_q:
                                nc.tensor.matmul(
                                    v_ps[:], xt2[:, md, u], wmat[:, md, GDIM : 2 * GDIM],
                                    start=(md == 0), stop=(md == 5), skip_group_check=True,
                                )
                        if not is_q:
                            # V: copy PSUM->SBUF on Act (frees v_ps without
                            # riding the DVE chain); ve added later on DVE
                            # from SBUF at 2x
                            nc.scalar.activation(vraw2[:, u], v_ps[:], COPY)
                        # K/Q: move PSUM -> SBUF bf16 (unnormalized)
                        nc.scalar.activation(
                            nat2[:, u].rearrange("p h d -> p (h d)"), k_ps[:], COPY
                        )

                    # ssq per (tile, head) from the PRE-rotary values
                    # (rotation preserves row norms); emitting it first lets
                    # the sqrt/recip chain overlap the rotary ops
                    sq2 = ap.tile([128, w, HPG, D], BF16, tag=f"sq{w}")
                    nc.vector.tensor_mul(out=sq2[:], in0=nat2[:], in1=nat2[:])
                    ssq = ap.tile([128, w * HPG], F32, tag=f"ssq{w}")
                    nc.vector.tensor_reduce(
                        out=ssq[:],
                        in_=sq2[:].rearrange("p u h d -> p (u h) d"),
                        axis=mybir.AxisListType.X,
                        op=mybir.AluOpType.add,
                    )

                    if not is_q:
                        # deferred V = v_raw + lam1*ve (all-bf16 SBUF, 2x)
                        nc.vector.tensor_add(
                            out=(Vq if DEDUP else Vn)[
                                :, w * tp : w * (tp + 1), :
                            ],
                            in0=vraw2[:], in1=vet2[:],
                        )

                    # rotary on each tile
                    ctab = cosq_sb if is_q else cosk_sb
                    stab = sinqpm_sb if is_q else sinkpm_sb
                    for u in range(w):
                        ti = w * tp + u
                        _rotary_u(nc, rot, nat2[:, u], ctab[:, ti], stab[:, ti])
                    # b = 1/sqrt(mean+eps); for Q fold ATTN_SCALE^2 into eps trick
                    bsc = ap.tile([128, w * HPG], F32, tag=f"bsc{w}")
                    if is_q:
                        s2 = ATTN_SCALE * ATTN_SCALE
                        nc.scalar.activation(
                            bsc[:], ssq[:], SQRT, bias=eps_q[:], scale=1.0 / (D * s2)
                        )
                    else:
                        nc.scalar.activation(
                            bsc[:], ssq[:], SQRT, bias=eps_k[:], scale=1.0 / D
                        )
                    bscb = ap.tile([128, w * HPG], BF16, tag=f"bscb{w}")
                    with nc.allow_low_precision(
                        reason="1/rms scale in bf16; rows are bf16 anyway"
                    ):
                        nc.vector.reciprocal(bscb[:], bsc[:])
                    # scale rows in place: one all-bf16 op (2x DVE mode)
                    # (K carries 1/rms, Q carries ATTN_SCALE/rms)
                    nc.vector.tensor_mul(
                        out=nat2[:].rearrange("p u h d -> p (u h) d"),
                        in0=nat2[:].rearrange("p u h d -> p (u h) d"),
                        in1=bscb[:, :, None].to_broadcast((128, w * HPG, D)),
                    )

                    # transposes delayed (PE pipelining: don't head-of-line
                    # block the next group's projections)
                    pending_tr.append((nat2, is_q, tp, w))

                for tp in range(NKVP):
                    proj_pair(tp, is_q=False)
                    if len(pending_tr) > 2:
                        flush_tr(1)
                if DEDUP:
                    flush_tr()  # KTq/Vq complete before the gather
                    KB = HPG * 128 * NKT  # bf16 elems per core (K^T quarter)
                    VB = NKT * GDIM  # bf16 elems per core (V quarter)
                    with tc.tile_pool(name="ccd", bufs=1, space="DRAM") as ccd:
                        BLOB = KB + VB
                        cc_in = ccd.tile([128, BLOB], BF16, tag="cc_in")
                        cc_out = ccd.tile([4, 128, BLOB], BF16, tag="cc_out")
                        nc.sync.dma_start(
                            cc_in[:, 0:KB], KTq[:].rearrange("p h t -> p (h t)")
                        )
                        nc.scalar.dma_start(
                            cc_in[:, KB : KB + VB],
                            Vq[:].rearrange("p t g -> p (t g)"),
                        )
                        groups = [[0, 1, 2, 3], [4, 5, 6, 7]]
                        if not sim_cc:
                            nc.gpsimd.collective_compute(
                                "AllGather", mybir.AluOpType.bypass,
                                replica_groups=groups,
                                ins=[cc_in.opt()], outs=[cc_out.opt()],
                            )
                        # Q projections overlap the gather; produce the
                        # second 512 rows first (attention's big chunk B
                        # reads qoff 512:1024, so it can start after two
                        # pairs instead of four)
                        for tp in (2, 3, 0, 1):
                            proj_pair(tp, is_q=True)
                            if len(pending_tr) > 2:
                                flush_tr(1)
                        flush_tr()
                        # assemble full K^T, V; slot i holds quarter QOF[i]
                        # (remap: core s computes quarter 3-s so low k-tiles
                        # land earliest for the high-s cores)
                        for q in range(4):
                            i = QOF.index(q)
                            nc.sync.dma_start(
                                KT[:, :, 1024 * q : 1024 * (q + 1)],
                                cc_out[i, :, 0:KB].rearrange(
                                    "p (h t) -> p h t", h=HPG
                                ),
                            )
                            nc.scalar.dma_start(
                                Vn[:, NKT * q : NKT * (q + 1), :],
                                cc_out[i, :, KB : KB + VB].rearrange(
                                    "p (t g) -> p t g", t=NKT
                                ),
                            )
                else:
                    for tp in range(NQ // 2):
                        proj_pair(tp, is_q=True)
                        if len(pending_tr) > 1:
                            flush_tr(1)
                    flush_tr()

            if DEBUG:
                nc.gpsimd.dma_start(dbg_kt[:], KT[:])
                nc.gpsimd.dma_start(dbg_qt[:], QT[:])
                nc.gpsimd.dma_start(dbg_vn[:], Vn[:])
            nc.gpsimd.dma_start(cproj_sb[:], cprojT[:])

            # ================= Phase D =================
            def cproj(s):
                with (
                    tc.tile_pool(name=f"op{s}", bufs=3) as op,
                    tc.tile_pool(name=f"psO{s}", bufs=3, space="PSUM") as psO,
                ):
                    for m in range(NQ):
                        o_sb = op.tile([128, DIM], BF16, tag="o_sb")
                        for oc in range(2):
                            o_ps = psO.tile([128, GDIM], F32, tag="o_ps")
                            for h in range(HPG):
                                nc.tensor.matmul(
                                    o_ps[:],
                                    Ysb[:, h, m * 128 : (m + 1) * 128],
                                    cproj_sb[:, h, oc * GDIM : (oc + 1) * GDIM],
                                    start=(h == 0), stop=(h == 2),
                                    skip_group_check=True,
                                )
                            nc.scalar.activation(
                                o_sb[:, oc * GDIM : (oc + 1) * GDIM], o_ps[:], COPY
                            )
                        nc.sync.dma_start(y_out[m * 128 : (m + 1) * 128, :], o_sb[:])

            # ================= Phase C: attention (per-core variant) ======
            def attention(s):
                with (
                    tc.tile_pool(name=f"ep{s}", bufs=6) as ep,
                    tc.tile_pool(name=f"rp{s}", bufs=2) as rp,
                    tc.tile_pool(name=f"psS{s}", bufs=4, space="PSUM") as psS,
                    tc.tile_pool(name=f"psY{s}", bufs=2, space="PSUM") as psY,
                    tc.tile_pool(name=f"psD{s}", bufs=2, space="PSUM") as psD,
                ):
                    # paired causal chunks (big first: small chunk is the tail)
                    chunks = [(512, 32 - 4 * s), (0, 4 * s + 4)]
                    qown = QOF[s]  # the quarter this core projected locally
                    for h in range(HPG):
                        for qoff, nk in chunks:
                            y_ps = psY.tile([128, 512], F32, tag="y")
                            d_ps = psD.tile([1, 512], F32, tag="d")

                            def score_exp(k):
                                i = k - (nk - 4)  # diagonal index when >= 0
                                qlo = 128 * i if i > 0 else 0
                                local = DEDUP and NKT * qown <= k < NKT * (qown + 1)
                                if local:  # pre-gather SBUF copies of own quarter
                                    kl = k - NKT * qown
                                    kt_src = KTq[:, h, kl * 128 : (kl + 1) * 128]
                                else:
                                    kt_src = KT[:, h, k * 128 : (k + 1) * 128]
                                s_ps = psS.tile([128, 512], F32, tag="s")
                                nc.tensor.matmul(
                                    s_ps[:, qlo:512],
                                    kt_src,
                                    QT[:, h, qoff + qlo : qoff + 512],
                                    start=True, stop=True, skip_group_check=True,
                                )
                                E = ep.tile([128, 512], BF16, tag="E")
                                nc.scalar.activation(
                                    E[:, qlo:512], s_ps[:, qlo:512], EXP
                                )
                                if i >= 0:
                                    # zero E in the diagonal block where k > q
                                    nc.gpsimd.affine_select(
                                        out=E[:, 128 * i : 128 * (i + 1)],
                                        in_=E[:, 128 * i : 128 * (i + 1)],
                                        compare_op=mybir.AluOpType.is_ge,
                                        fill=0.0, base=0,
                                        pattern=[[1, 128]], channel_multiplier=-1,
                                    )
                                return E, qlo

                            def dy(k, E, qlo, first, last):
                                local = DEDUP and NKT * qown <= k < NKT * (qown + 1)
                                if local:
                                    v_src = Vq[:, k - NKT * qown, h * D : (h + 1) * D]
                                else:
                                    v_src = Vn[:, k, h * D : (h + 1) * D]
                                nc.tensor.matmul(
                                    d_ps[:, qlo:512], ones_col[:], E[:, qlo:512],
                                    start=first, stop=last, skip_group_check=True,
                                )
                                nc.tensor.matmul(
                                    y_ps[:, qlo:512], v_src, E[:, qlo:512],
                                    start=first, stop=last, skip_group_check=True,
                                )

                            # k order: own (pre-gather) quarter first, then the
                            # rest; accumulation order is irrelevant, but the
                            # first emitted tile must cover the full q range
                            loc = [k for k in range(nk)
                                   if DEDUP and NKT * qown <= k < NKT * (qown + 1)]
                            rest = [k for k in range(nk) if k not in loc]
                            order = loc + rest
                            assert order[0] - (nk - 4) <= 0, "first tile not full"
                            # software pipeline depth 2: exp(k+1..k+2) overlap
                            # the denominator/AV matmuls of tile k on PE
                            fifo = []
                            for pos, k in enumerate(order):
                                fifo.append((k, *score_exp(k)))
                                if len(fifo) > 2:
                                    kk, E, qlo = fifo.pop(0)
                                    p0 = order.index(kk)
                                    dy(kk, E, qlo, p0 == 0, p0 == nk - 1)
                            while fifo:
                                kk, E, qlo = fifo.pop(0)
                                p0 = order.index(kk)
                                dy(kk, E, qlo, p0 == 0, p0 == nk - 1)
                            recip = rp.tile([1, 512], F32R, tag="recip")
                            with nc.allow_low_precision(
                                reason="1/denom as f32r; ~1e-4 uniform scale wobble"
                            ):
                                nc.vector.reciprocal(recip[:], d_ps[:])
                            bc = rp.tile([128, 512], F32R, tag="bc")
                            nc.gpsimd.partition_broadcast(bc[:], recip[0:1, :])
                            nc.vector.tensor_mul(
                                out=Ysb[:, h, qoff : qoff + 512], in0=y_ps[:], in1=bc[:]
                            )
                if DEBUG:
                    nc.gpsimd.dma_start(dbg_ys[:], Ysb[:])
                cproj(s)

            if variant is not None:
                attention(variant)
            else:
                # core-variant register (s = core % 4), loaded late so the
                # projection phase isn't gated on the all-engine snap
                tmp = nc.alloc_registers("tmp_svar", mybir.ALL_ENGINES)
                nc.regs_load(tmp, svar_t[0:1, 0:1])
                sv = nc.snap(tmp, donate=True, min_val=0, max_val=3)
                with tc.If(sv == 0) as c0:
                    attention(0)
                with c0.Else():
                    with tc.If(sv == 1) as c1:
                        attention(1)
                    with c1.Else():
                        with tc.If(sv == 2) as c2:
                            attention(2)
                        with c2.Else():
                            attention(3)

    nc.finalize()
    return nc


def _host_prep(x, ve, qkv_w, lambdas, c_proj_w):
    """Build the 8 per-core input maps."""
    x2d = np.ascontiguousarray(x.reshape(T, DIM), dtype=np.float32)
    xT = np.ascontiguousarray(x2d.T)
    ve2 = ve.reshape(T, H, D).astype(np.float32)
    lam0, lam1 = float(lambdas[0]), float(lambdas[1])
    wq_all, wk_all, wv_all = qkv_w[0], qkv_w[1], qkv_w[2]  # [768, 768] each

    t = np.arange(T, dtype=np.float32)
    af = (1.0 / 1024.0) ** np.linspace(0.0, 1.0, 32, dtype=np.float32)
    theta = t[:, None] * af[None, :]
    cos_t = np.cos(theta).astype(np.float32)  # [T, 32]
    sin_t = np.sin(theta).astype(np.float32)
    sin_pm = np.concatenate([sin_t, -sin_t], axis=1)  # [T, 64]

    ident = np.eye(128, dtype=NPBF16)
    ones_col = np.ones((128, 1), dtype=NPBF16)

    def pack_xT(m):  # [768, t] -> [t/256, 128, 6, 2, 128] bf16
        t = m.shape[1]
        return np.ascontiguousarray(
            m.reshape(6, 128, t // 256, 2, 128).transpose(2, 1, 0, 3, 4)
        ).astype(NPBF16)

    def pack_rows(m):  # [t, d] -> [t/256, 128, 2, d]
        t, d = m.shape
        return np.ascontiguousarray(m.reshape(t // 256, 2, 128, d).transpose(0, 2, 1, 3))

    def pack_tab(m):  # [t, c] -> [128, t/128, c] bf16
        t, c = m.shape
        return np.ascontiguousarray(
            m.reshape(t // 128, 128, c).transpose(1, 0, 2)
        ).astype(NPBF16)

    xT_packed = pack_xT(xT)
    cosk_p = pack_tab(cos_t)
    sinkpm_p = pack_tab(sin_pm)
    # per-head-group tensors computed once, shared by the 4 cores of a group
    per_g = []
    for g in range(2):
        hsl = slice(g * GDIM, (g + 1) * GDIM)
        wkv = np.concatenate([wk_all[hsl], lam0 * wv_all[hsl]], axis=0)
        per_g.append(
            {
                "wkv": np.ascontiguousarray(
                    wkv.T.astype(np.float32).reshape(6, 128, 768).transpose(1, 0, 2)
                ).astype(NPBF16),
                "wq": np.ascontiguousarray(
                    wq_all[hsl].T.astype(np.float32).reshape(6, 128, GDIM).transpose(1, 0, 2)
                ).astype(NPBF16),
                "cprojT": np.ascontiguousarray(
                    c_proj_w[:, hsl].T.astype(np.float32).reshape(HPG, 128, DIM).transpose(1, 0, 2)
                ).astype(NPBF16),
                "ve": (lam1 * ve2[:, g * HPG : (g + 1) * HPG, :])
                .reshape(T, GDIM)
                .astype(NPBF16),
            }
        )
    per_s = []
    for s in range(4):
        qrows = np.r_[512 * s : 512 * (s + 1), 512 * (7 - s) : 512 * (8 - s)]
        per_s.append(
            {
                "xqTt": pack_xT(np.ascontiguousarray(xT[:, qrows])),
                "cosq": pack_tab(np.ascontiguousarray(cos_t[qrows])),
                "sinqpm": pack_tab(np.ascontiguousarray(sin_pm[qrows])),
                "svar": np.array([[s]], dtype=np.uint32),
            }
        )
    in_maps = []
    for c in range(8):
        g, s = divmod(c, 4)
        if DEDUP:  # K/V projection: this core's assigned quarter (remapped)
            qq = QOF[s]
            kvp = slice(4 * qq, 4 * (qq + 1))  # of 16 tile-pairs
            kvt = slice(8 * qq, 8 * (qq + 1))  # of 32 tiles
            kvr = slice(1024 * qq, 1024 * (qq + 1))  # of 4096 rows
        else:
            kvp = kvt = slice(None)
            kvr = slice(0, T)
        pg, ps = per_g[g], per_s[s]
        in_maps.append(
            {
                "xTt": xT_packed[kvp],
                "xqTt": ps["xqTt"],
                "wkv": pg["wkv"],
                "wq": pg["wq"],
                "vek": pack_rows(np.ascontiguousarray(pg["ve"][kvr])),
                "cosk": np.ascontiguousarray(cosk_p[:, kvt]),
                "sinkpm": np.ascontiguousarray(sinkpm_p[:, kvt]),
                "cosq": ps["cosq"],
                "sinqpm": ps["sinqpm"],
                "cprojT": pg["cprojT"],
                "ident": ident,
                "ones_col": ones_col,
                "svar": ps["svar"],
            }
        )
    return in_maps


def _make_runner(nc, n_cores=8):
    """Compile the SPMD NEFF once; return a callable taking per-core input
    maps (run_bass_kernel_spmd under axon re-traces jax.jit every call)."""
    import jax
    from jax.sharding import Mesh, NamedSharding, PartitionSpec
    from jax.experimental.shard_map import shard_map
    import concourse.bass2jax as b2j

    b2j.install_neuronx_cc_hook()
    partition_name = nc.partition_id_tensor.name if nc.partition_id_tensor else None
    in_names, out_names, out_avals, zero_outs = [], [], [], []
    for alloc in nc.m.functions[0].allocations:
        if not isinstance(alloc, mybir.MemoryLocationSet):
            continue
        name = alloc.memorylocations[0].name
        if alloc.kind == "ExternalInput":
            if name != partition_name:
                in_names.append(name)
        elif alloc.kind == "ExternalOutput":
            shape = tuple(alloc.tensor_shape)
            dtype = mybir.dt.np(alloc.dtype)
            out_names.append(name)
            out_avals.append(jax.core.ShapedArray(shape, dtype))
            zero_outs.append(np.zeros((n_cores * shape[0], *shape[1:]), dtype))
    all_in = list(in_names) + list(out_names)
    if partition_name is not None:
        all_in.append(partition_name)

    def _body(*args):
        operands = list(args)
        if partition_name is not None:
            operands.append(b2j.partition_id_tensor())
        outs = b2j._bass_exec_p.bind(
            *operands,
            out_avals=tuple(out_avals),
            in_names=tuple(all_in),
            out_names=tuple(out_names),
            lowering_input_output_aliases=(),
            sim_require_finite=True,
            sim_require_nnan=True,
            nc=nc,
        )
        return tuple(outs)

    devices = jax.devices()[:n_cores]
    mesh = Mesh(np.asarray(devices), ("core",))
    nspec = NamedSharding(mesh, PartitionSpec("core"))
    nin = len(in_names)
    sharded = jax.jit(
        shard_map(
            _body, mesh=mesh,
            in_specs=(PartitionSpec("core"),) * (nin + len(out_names)),
            out_specs=(PartitionSpec("core"),) * len(out_names),
            check_rep=False,
        ),
        keep_unused=True,
    )
    dev_zero = [jax.device_put(z, nspec) for z in zero_outs]

    def run_maps(in_maps):
        concat = [
            np.concatenate([np.asarray(m[name]) for m in in_maps], axis=0)
            for name in in_names
        ]
        dev_in = [jax.device_put(a, nspec) for a in concat]
        outs = sharded(*dev_in, *dev_zero)
        outs = [np.asarray(o) for o in outs]
        return [
            {
                name: outs[i].reshape(n_cores, *out_avals[i].shape)[c]
                for i, name in enumerate(out_names)
            }
            for c in range(n_cores)
        ]

    return run_maps


class _Res:
    def __init__(self, results):
        self.results = results


def run(inputs, **run_kwargs):
    if "nc" not in _CACHE:
        _CACHE["nc"] = build_nc()
    nc = _CACHE["nc"]
    in_maps = _host_prep(
        inputs["x"], inputs["ve"], inputs["qkv_w"], inputs["lambdas"], inputs["c_proj_w"]
    )
    if run_kwargs:
        res = run_bass_kernel_spmd(nc, in_maps, core_ids=list(range(8)), **run_kwargs)
        results = res.results
    else:
        if "runner" not in _CACHE:
            _CACHE["runner"] = _make_runner(nc)
        results = _CACHE["runner"](in_maps)
        res = _Res(results)
    out = np.zeros((T, DIM), dtype=np.float32)
    for c, r in enumerate(results):
        s = c % 4
        y = r["y"]
        out[512 * s : 512 * (s + 1)] += y[:512]
        out[512 * (7 - s) : 512 * (8 - s)] += y[512:]
    return out.reshape(1, T, DIM), res


def kernel(**inputs):
    out, _ = run(inputs)
    return out
